# revision 1
# baseline (speedup 1.0000x reference)
"""MoE ConditionalFeedForward kernel for 8 trn2 NeuronCores.

Strategy: expert parallelism. E=8 experts == 8 cores, so core k owns expert k's
weights (w1[k], w3[k], w2[k]) and processes exactly the (token, slot) pairs
routed to expert k. Routing/gather/scatter run on host; the heavy compute
(3 x C x D x I MACs per core over 1.1 GB of weights) runs on device.

Device math per core (C = padded token capacity, D=2048, I=5632):
  phase 1: hT[i, c] = silu(sum_d w1T[d,i] xT[d,c]) * (sum_d w3T[d,i] xT[d,c])
           (PE matmuls with d on partitions; w1/w3 pre-transposed on host)
  phase 2: y[c, d]  = sum_i hT[i, c] * w2[i, d]
           (PE matmuls with i on partitions; w2 in natural layout)

All weights/activations stream as bf16 (1 PE cycle/row vs 4 for f32; half the
HBM traffic); PSUM accumulation is f32 and the output is f32.
"""

import numpy as np
import ml_dtypes

BF16 = ml_dtypes.bfloat16

# Problem dims (hardcoded per contract; kernel.py must be self-contained).
T, A, E, D, I = 1024, 2, 8, 2048, 5632
N_CORES = 8

_BUILD_CACHE = {}


def _pick_groups(ib):
    """Blocks-per-DMA for the phase-1 (w1/w3) and phase-2 (w2) weight streams."""
    g1 = 2 if ib % 2 == 0 else 1
    g2 = 4 if ib % 4 == 0 else (2 if ib % 2 == 0 else 1)
    return g1, g2


def _pick_npass(d):
    """Split phase 2's D dim into npass passes so the live yT PSUM accumulator
    tags ((d/npass)/128 of them) fit in 8 banks. Prefer double-buffered
    (bufs=2) tags so consecutive passes overlap, then the fewest passes."""
    for bufs in (2, 1):
        for npass in (1, 2, 4, 8, 16):
            ndc = d // npass // 128
            if d % npass == 0 and (d // npass) % 128 == 0 and ndc * bufs <= 8:
                return npass, bufs
    raise ValueError(f"no valid npass for d={d}")


def _build(cap, d=D, i_dim=I):
    """Build + compile the per-core Bass program for token capacity `cap`."""
    key = (cap, d, i_dim)
    if key in _BUILD_CACHE:
        return _BUILD_CACHE[key]

    import concourse.mybir as mybir
    import concourse.tile as tile
    from concourse import bacc

    dt = mybir.dt
    WDT = dt.bfloat16
    F32 = dt.float32

    db = d // 128          # d-chunks (contraction of phase 1)
    ib = i_dim // 128      # i-blocks (contraction of phase 2)
    g1, g2 = _pick_groups(ib)
    ng1, ng2 = ib // g1, ib // g2
    assert cap % 32 == 0 and cap <= 512
    npass, ps_bufs = _pick_npass(d)
    w = d // npass         # output columns per phase-2 pass
    nw = w // 512          # 512-col chunks per pass

    nc = bacc.Bacc("TRN2", target_bir_lowering=False, debug=False,
                   num_devices=N_CORES)

    xgt = nc.dram_tensor("xgt", [128, db * cap], WDT, kind="ExternalInput").ap()
    w1d = nc.dram_tensor("w1d", [ng1, 128, g1 * db * 128], WDT,
                         kind="ExternalInput").ap()
    w3d = nc.dram_tensor("w3d", [ng1, 128, g1 * db * 128], WDT,
                         kind="ExternalInput").ap()
    w2d = nc.dram_tensor("w2d", [npass, 128, ib * w], WDT,
                         kind="ExternalInput").ap()
    # output is y transposed ([D, cap]) so phase 2 can make w2's d-columns the
    # stationary M dim (divides exactly -> no M padding) and write the PSUM
    # [d_block, c] tiles out contiguously; the host untransposes for free.
    yt = nc.dram_tensor("yt", [d, cap], F32, kind="ExternalOutput").ap()

    with tile.TileContext(nc) as tc:
        with (
            tc.tile_pool(name="xpool", bufs=1) as xpool,
            tc.tile_pool(name="w1pool", bufs=3) as w1pool,
            tc.tile_pool(name="w3pool", bufs=3) as w3pool,
            tc.tile_pool(name="w2pool", bufs=3) as w2pool,
            tc.tile_pool(name="hpool", bufs=1) as hpool,
            tc.tile_pool(name="spool", bufs=2) as spool,
            tc.tile_pool(name="opool", bufs=4) as opool,
        ):
            xg = xpool.tile([128, db * cap], WDT)
            # chunked so the first matmuls don't wait on the whole transfer
            xq = max(1, db // 4) * cap
            for q0 in range(0, db * cap, xq):
                nc.sync.dma_start(xg[:, q0:q0 + xq], xgt[:, q0:q0 + xq])
            h = hpool.tile([128, ib * cap], WDT)

            # ---- phase 1: hT blocks ----
            with tc.tile_pool(name="psA", bufs=2, space="PSUM") as psA:
                for g in range(ng1):
                    wt1 = w1pool.tile([128, g1 * db * 128], WDT, tag="w1")
                    nc.sync.dma_start(wt1[:], w1d[g])
                    wt3 = w3pool.tile([128, g1 * db * 128], WDT, tag="w3")
                    nc.sync.dma_start(wt3[:], w3d[g])
                    for s in range(g1):
                        b = g * g1 + s
                        ps1 = psA.tile([128, cap], F32, tag="ps1")
                        ps3 = psA.tile([128, cap], F32, tag="ps3")
                        for do in range(db):
                            lo = (s * db + do) * 128
                            nc.tensor.matmul(
                                ps1[:], wt1[:, lo:lo + 128],
                                xg[:, do * cap:(do + 1) * cap],
                                start=(do == 0), stop=(do == db - 1))
                        for do in range(db):
                            lo = (s * db + do) * 128
                            nc.tensor.matmul(
                                ps3[:], wt3[:, lo:lo + 128],
                                xg[:, do * cap:(do + 1) * cap],
                                start=(do == 0), stop=(do == db - 1))
                        sig = spool.tile([128, cap], F32, tag="sig")
                        nc.scalar.activation(
                            sig[:], ps1[:],
                            mybir.ActivationFunctionType.Sigmoid)
                        m1 = spool.tile([128, cap], F32, tag="m1")
                        nc.vector.tensor_mul(m1[:], sig[:], ps3[:])
                        nc.vector.tensor_mul(
                            h[:, b * cap:(b + 1) * cap], m1[:], ps1[:])

            # ---- phase 2: yT[d, c] = sum_b w2[b, d].T @ hT[b, c] ----
            # stationary = w2 128-column d-blocks (M=128 exact), moving = hT
            # (N=cap). Output tiles are yT blocks, accumulated over all i.
            ndc = w // 128                      # 128-col d-blocks per pass
            # w2 groups: ~8 i-blocks per DMA (1 MB) for full HBM efficiency
            gsz = 8
            groups = [(b0, min(gsz, ib - b0)) for b0 in range(0, ib, gsz)]
            with tc.tile_pool(name="psB", bufs=ps_bufs, space="PSUM") as psB:
                for ph in range(npass):
                    po = {}
                    for dc in range(ndc):
                        po[dc] = psB.tile([128, cap], F32, tag=f"yT{dc}",
                                          name=f"po{dc}")
                    for b0, nb in groups:
                        wt2 = w2pool.tile([128, gsz * w], WDT, tag="w2")
                        nc.sync.dma_start(wt2[:, :nb * w],
                                          w2d[ph][:, b0 * w:(b0 + nb) * w])
                        for s in range(nb):
                            b = b0 + s
                            for dc in range(ndc):
                                lo = s * w + dc * 128
                                nc.tensor.matmul(
                                    po[dc][:],
                                    wt2[:, lo:lo + 128],
                                    h[:, b * cap:(b + 1) * cap],
                                    start=(b == 0), stop=(b == ib - 1))
                    for dc in range(ndc):
                        ot = opool.tile([128, cap], F32, tag="ot")
                        nc.vector.tensor_copy(ot[:], po[dc][:])
                        nc.scalar.dma_start(
                            yt[ph * w + dc * 128:ph * w + dc * 128 + 128, :],
                            ot[:])

    nc.compile()
    _BUILD_CACHE[key] = nc
    return nc


def _pack_w13(wk, d=D, i_dim=I):
    """Host-side relayout of a [I, D] w1/w3 matrix into the pre-transposed
    phase-1 device layout (see _build)."""
    db, ib = d // 128, i_dim // 128
    g1, _ = _pick_groups(ib)
    ng1 = ib // g1
    # [g, s, i_in, do, di] -> [g, di, s, do, i_in]
    return np.ascontiguousarray(
        wk.reshape(ng1, g1, 128, db, 128).transpose(0, 4, 1, 3, 2)
    ).reshape(ng1, 128, g1 * db * 128)


def _pack_w2(w2k, npass, d=D, i_dim=I):
    """[I, D] -> [ph, i_in, b*w + dcol]: per-pass flat block-major layout so
    phase 2 can DMA any run of i-blocks as one big contiguous transfer."""
    ib = i_dim // 128
    w = d // npass
    # [b, i_in, ph, dcol] -> [ph, i_in, b, dcol]
    return np.ascontiguousarray(
        w2k.reshape(ib, 128, npass, w).transpose(2, 1, 0, 3)
    ).reshape(npass, 128, ib * w)


def _prepare(inputs):
    """Host routing + packing. Returns (nc, in_maps, scatter_info)."""
    x = np.asarray(inputs["x"])
    idx = np.asarray(inputs["expert_indices"])
    w1 = np.asarray(inputs["w1"])
    w2 = np.asarray(inputs["w2"])
    w3 = np.asarray(inputs["w3"])

    t, a = idx.shape
    d, i_dim = x.shape[1], w1.shape[1]
    db = d // 128

    # ---- host routing (the "all-to-all") ----
    flat = idx.reshape(-1).astype(np.int64)
    order = np.argsort(flat, kind="stable")          # pair ids grouped by expert
    counts = np.bincount(flat, minlength=E)
    starts = np.concatenate([[0], np.cumsum(counts)])
    cap = max(128, int(-(-counts.max() // 32) * 32))  # round up to mult of 32
    assert cap <= 512, f"capacity {cap} > 512 unsupported"
    npass, _ = _pick_npass(d)

    nc = _build(cap, d, i_dim)

    x_bf = x.astype(BF16)
    in_maps = []
    for k in range(E):
        sel = order[starts[k]:starts[k + 1]] // a      # token ids for expert k
        xg = np.zeros((cap, d), BF16)
        xg[:len(sel)] = x_bf[sel]
        # [c, d] -> [di, do, c]
        xgt = np.ascontiguousarray(
            xg.T.reshape(db, 128, cap).transpose(1, 0, 2)
        ).reshape(128, db * cap)
        w1d_ = _pack_w13(w1[k].astype(BF16), d, i_dim)
        w3d_ = _pack_w13(w3[k].astype(BF16), d, i_dim)
        w2d_ = _pack_w2(w2[k].astype(BF16), npass, d, i_dim)
        in_maps.append({"xgt": xgt, "w1d": w1d_, "w3d": w3d_, "w2d": w2d_})

    return nc, in_maps, (t, a, d, order, counts, starts)


def _scatter(results, scatter_info):
    t, a, d, order, counts, starts = scatter_info
    out_flat = np.zeros((t * a, d), np.float32)
    for k in range(E):
        n_k = int(counts[k])
        if n_k:
            out_flat[order[starts[k]:starts[k] + n_k]] = \
                results[k]["yt"][:, :n_k].T
    return out_flat.reshape(t, a, d)


def kernel(**inputs):
    from concourse.bass_utils import run_bass_kernel_spmd

    nc, in_maps, scatter_info = _prepare(inputs)
    res = run_bass_kernel_spmd(nc, in_maps, core_ids=list(range(N_CORES)))
    return _scatter(res.results, scatter_info)



# revision 3
# speedup vs baseline: 1.1254x; 1.1254x over previous
"""MoE ConditionalFeedForward kernel for 8 trn2 NeuronCores.

Strategy: expert parallelism with 2-way intermediate (I) splitting for load
balance. The 8 experts are ranked by routed-token count and paired
heaviest-with-lightest into 4 groups; the two cores of group g each own HALF
the I-rows (22 of 44 128-row blocks) of BOTH experts in the group and process
ALL tokens routed to them. Each core therefore streams exactly 44 block-rows
of w1/w3/w2 (the same HBM traffic as one full expert) while its matmul column
count is bounded by max-heavy + max-light counts instead of the global max.
Token (t, slot) pairs whose two slots hit the same expert are deduplicated
(computed once, scattered twice).

Per core, slice s in {heavy, light} with capacity C_s:
  phase 1: hT[i, c] = silu(sum_d w1T[d,i] xT[d,c]) * (sum_d w3T[d,i] xT[d,c])
           for the 22 owned i-blocks (PE matmuls, d on partitions)
  phase 2: yT[d, c] = sum_{i in owned} hT[i, c] * w2[i, d]  (partial sum)
The two partial yT of an expert are summed on the host (f32) and scattered.

All weights/activations stream as bf16; PSUM accumulation is f32.
"""

import numpy as np
import ml_dtypes

BF16 = ml_dtypes.bfloat16

# Problem dims (hardcoded per contract; kernel.py must be self-contained).
T, A, E, D, I = 1024, 2, 8, 2048, 5632
N_CORES = 8
DB = D // 128          # 16 d-chunks (phase-1 contraction)
IB = I // 128          # 44 i-blocks total per expert
HB = IB // 2           # 22 i-blocks per core slice
NPASS = 8              # phase-2 passes over D
W = D // NPASS         # 256 output columns per phase-2 pass
NDC = W // 128         # 2 128-col d-blocks per pass
GSZ = 8                # w2 i-blocks per DMA group

_BUILD_CACHE = {}


def _pad4(n):
    return max(4, -(-int(n) // 4) * 4)


def _build(C1, C2):
    """Build + compile the per-core Bass program for slice capacities C1, C2."""
    key = (C1, C2)
    if key in _BUILD_CACHE:
        return _BUILD_CACHE[key]

    import concourse.mybir as mybir
    import concourse.tile as tile
    from concourse import bacc

    dt = mybir.dt
    WDT = dt.bfloat16
    F32 = dt.float32

    assert C1 <= 512 and C2 <= C1

    nc = bacc.Bacc("TRN2", target_bir_lowering=False, debug=False,
                   num_devices=N_CORES)

    xa_t = nc.dram_tensor("xga", [128, DB * C1], WDT, kind="ExternalInput").ap()
    xb_t = nc.dram_tensor("xgb", [128, DB * C2], WDT, kind="ExternalInput").ap()
    w1a_t = nc.dram_tensor("w1a", [128, HB * DB * 128], WDT,
                           kind="ExternalInput").ap()
    w3a_t = nc.dram_tensor("w3a", [128, HB * DB * 128], WDT,
                           kind="ExternalInput").ap()
    w1b_t = nc.dram_tensor("w1b", [128, HB * DB * 128], WDT,
                           kind="ExternalInput").ap()
    w3b_t = nc.dram_tensor("w3b", [128, HB * DB * 128], WDT,
                           kind="ExternalInput").ap()
    w2a_t = nc.dram_tensor("w2a", [NPASS, 128, HB * W], WDT,
                           kind="ExternalInput").ap()
    w2b_t = nc.dram_tensor("w2b", [NPASS, 128, HB * W], WDT,
                           kind="ExternalInput").ap()
    # outputs are y transposed ([D, C]) partial sums; host adds + untransposes.
    ya_t = nc.dram_tensor("yta", [D, C1], F32, kind="ExternalOutput").ap()
    yb_t = nc.dram_tensor("ytb", [D, C2], F32, kind="ExternalOutput").ap()

    slices = [(C1, xa_t, w1a_t, w3a_t, w2a_t, ya_t),
              (C2, xb_t, w1b_t, w3b_t, w2b_t, yb_t)]

    with tile.TileContext(nc) as tc:
        with (
            tc.tile_pool(name="xpool", bufs=1) as xpool,
            tc.tile_pool(name="w1pool", bufs=4) as w1pool,
            tc.tile_pool(name="w3pool", bufs=4) as w3pool,
            tc.tile_pool(name="w2pool", bufs=4) as w2pool,
            tc.tile_pool(name="hpool", bufs=1) as hpool,
            tc.tile_pool(name="spool", bufs=2) as spool,
            tc.tile_pool(name="opool", bufs=4) as opool,
            tc.tile_pool(name="ps", bufs=2, space="PSUM") as ps,
        ):
            xg = {}
            for s, (C, x_t, *_r) in enumerate(slices):
                xg[s] = xpool.tile([128, DB * C], WDT, tag=f"x{s}",
                                   name=f"xg{s}")

            for s, (C, x_t, w1_t, w3_t, w2_t, y_t) in enumerate(slices):
                h = hpool.tile([128, HB * C], WDT, tag=f"h{s}")

                # ---- phase 1 ----
                # warmup: tiny first weight groups so the first matmul is
                # gated on ~0.25 MB of DMA instead of a full 1 MB tile.
                groups = ([1, 1] + [2] * 10) if s == 0 else [2] * 11
                b0 = 0
                for gi, nb in enumerate(groups):
                    wt1 = w1pool.tile([128, 2 * DB * 128], WDT, tag="w1")
                    span = nb * DB * 128
                    if s == 0 and gi == 0:
                        # chunk the very first w1 group: the do=0 matmul only
                        # needs the first half.
                        hspan = span // 2
                        nc.sync.dma_start(wt1[:, :hspan], w1_t[:, :hspan])
                        # x for slice 0, first 4 d-chunks right behind it
                        nc.sync.dma_start(xg[0][:, :4 * C], x_t[:, :4 * C])
                        nc.sync.dma_start(wt1[:, hspan:span],
                                          w1_t[:, hspan:span])
                    else:
                        nc.sync.dma_start(
                            wt1[:, :span],
                            w1_t[:, b0 * DB * 128:(b0 + nb) * DB * 128])
                    wt3 = w3pool.tile([128, 2 * DB * 128], WDT, tag="w3")
                    nc.sync.dma_start(
                        wt3[:, :span],
                        w3_t[:, b0 * DB * 128:(b0 + nb) * DB * 128])
                    if s == 0 and gi == 0:
                        # rest of slice-0 x
                        for q0 in range(4 * C, DB * C, 6 * C):
                            q1 = min(q0 + 6 * C, DB * C)
                            nc.sync.dma_start(xg[0][:, q0:q1], x_t[:, q0:q1])
                    if s == 0 and gi == 2:
                        # slice-1 x, issued early so it streams during
                        # slice-0 compute (needed ~150us later).
                        C2_, xb = slices[1][0], slices[1][1]
                        for q0 in range(0, DB * C2_, 8 * C2_):
                            q1 = min(q0 + 8 * C2_, DB * C2_)
                            nc.sync.dma_start(xg[1][:, q0:q1], xb[:, q0:q1])

                    for sb in range(nb):
                        b = b0 + sb
                        ps1 = ps.tile([128, C1], F32, tag="ps1")
                        ps3 = ps.tile([128, C1], F32, tag="ps3")
                        for do in range(DB):
                            lo = (sb * DB + do) * 128
                            nc.tensor.matmul(
                                ps1[:, :C], wt1[:, lo:lo + 128],
                                xg[s][:, do * C:(do + 1) * C],
                                start=(do == 0), stop=(do == DB - 1))
                        for do in range(DB):
                            lo = (sb * DB + do) * 128
                            nc.tensor.matmul(
                                ps3[:, :C], wt3[:, lo:lo + 128],
                                xg[s][:, do * C:(do + 1) * C],
                                start=(do == 0), stop=(do == DB - 1))
                        sig = spool.tile([128, C1], F32, tag="sig")
                        nc.scalar.activation(
                            sig[:, :C], ps1[:, :C],
                            mybir.ActivationFunctionType.Sigmoid)
                        m1 = spool.tile([128, C1], F32, tag="m1")
                        nc.vector.tensor_mul(m1[:, :C], sig[:, :C], ps3[:, :C])
                        nc.vector.tensor_mul(
                            h[:, b * C:(b + 1) * C], m1[:, :C], ps1[:, :C])
                    b0 += nb

                # ---- phase 2: yT[d, c] = sum_b w2[b, d].T @ hT[b, c] ----
                w2groups = [(g0, min(GSZ, HB - g0)) for g0 in range(0, HB, GSZ)]
                for ph in range(NPASS):
                    po = {}
                    for dc in range(NDC):
                        po[dc] = ps.tile([128, C1], F32, tag=f"y{dc}",
                                         name=f"po{dc}")
                    for g0, nb in w2groups:
                        wt2 = w2pool.tile([128, GSZ * W], WDT, tag="w2")
                        nc.sync.dma_start(wt2[:, :nb * W],
                                          w2_t[ph][:, g0 * W:(g0 + nb) * W])
                        for sb in range(nb):
                            b = g0 + sb
                            for dc in range(NDC):
                                lo = sb * W + dc * 128
                                nc.tensor.matmul(
                                    po[dc][:, :C],
                                    wt2[:, lo:lo + 128],
                                    h[:, b * C:(b + 1) * C],
                                    start=(b == 0), stop=(b == HB - 1))
                    for dc in range(NDC):
                        ot = opool.tile([128, C1], F32, tag="ot")
                        nc.vector.tensor_copy(ot[:, :C], po[dc][:, :C])
                        nc.scalar.dma_start(
                            y_t[ph * W + dc * 128:ph * W + dc * 128 + 128, :],
                            ot[:, :C])

    nc.compile()
    _BUILD_CACHE[key] = nc
    return nc


def _pack13(wh):
    """[2816, 2048] w1/w3 half -> phase-1 layout [128, HB*DB*128]:
    col = (b*DB + do)*128 + i_in, partition = d_in."""
    return np.ascontiguousarray(
        wh.reshape(HB, 128, DB, 128).transpose(3, 0, 2, 1)
    ).reshape(128, HB * DB * 128)


def _pack2(wh):
    """[2816, 2048] w2 half -> phase-2 layout [NPASS, 128, HB*W]:
    per pass, col = b*W + j, partition = i_in."""
    return np.ascontiguousarray(
        wh.reshape(HB, 128, NPASS, W).transpose(2, 1, 0, 3)
    ).reshape(NPASS, 128, HB * W)


def _packx(x_bf, tokens, C):
    """Gather token rows of x (bf16) and lay out as [128, DB*C]:
    col = do*C + c, partition = d_in."""
    xp = np.zeros((C, D), BF16)
    xp[:len(tokens)] = x_bf[tokens]
    return np.ascontiguousarray(
        xp.reshape(C, DB, 128).transpose(2, 1, 0)
    ).reshape(128, DB * C)


def _prepare(inputs):
    """Host routing + packing. Returns (nc, in_maps, scatter_info)."""
    x = np.asarray(inputs["x"])
    idx = np.asarray(inputs["expert_indices"])
    w1 = np.asarray(inputs["w1"])
    w2 = np.asarray(inputs["w2"])
    w3 = np.asarray(inputs["w3"])

    t_n, a_n = idx.shape

    # ---- dedup + routing ----
    tt = np.repeat(np.arange(t_n), a_n)
    ee = idx.reshape(-1).astype(np.int64)
    keys = tt * E + ee
    uniq = np.unique(keys)                        # sorted (t, e) pairs
    ue = uniq % E
    ut = uniq // E
    order = np.argsort(ue, kind="stable")         # grouped by expert
    counts = np.bincount(ue, minlength=E)
    starts = np.concatenate([[0], np.cumsum(counts)])
    # concat-layout row of each unique pair, and the gather map for scatter
    col = np.empty(len(uniq), np.int64)
    col[order] = np.arange(len(uniq)) - starts[ue[order]]
    concat_row = starts[ue] + col
    gather_rows = concat_row[np.searchsorted(uniq, keys)]   # [T*A]

    # ---- heavy/light pairing ----
    rank = np.argsort(-counts, kind="stable")
    pairs = [(int(rank[i]), int(rank[7 - i])) for i in range(4)]
    C1 = _pad4(counts[rank[0]])
    C2 = _pad4(counts[rank[4]])
    tokens_of = {
        int(e): ut[order[starts[e]:starts[e] + counts[e]]] for e in range(E)
    }

    nc = _build(C1, C2)

    x_bf = x.astype(BF16)
    w1_bf = {}
    in_maps = [dict() for _ in range(N_CORES)]
    for g, (he, le) in enumerate(pairs):
        xa = _packx(x_bf, tokens_of[he], C1)
        xb = _packx(x_bf, tokens_of[le], C2)
        for half in range(2):
            c = 2 * g + half
            r0, r1 = half * (I // 2), (half + 1) * (I // 2)
            in_maps[c]["xga"] = xa
            in_maps[c]["xgb"] = xb
            in_maps[c]["w1a"] = _pack13(w1[he][r0:r1].astype(BF16))
            in_maps[c]["w3a"] = _pack13(w3[he][r0:r1].astype(BF16))
            in_maps[c]["w2a"] = _pack2(w2[he][r0:r1].astype(BF16))
            in_maps[c]["w1b"] = _pack13(w1[le][r0:r1].astype(BF16))
            in_maps[c]["w3b"] = _pack13(w3[le][r0:r1].astype(BF16))
            in_maps[c]["w2b"] = _pack2(w2[le][r0:r1].astype(BF16))

    scatter_info = (t_n, a_n, pairs, counts, starts, gather_rows, len(uniq))
    return nc, in_maps, scatter_info


def _scatter(results, scatter_info):
    t_n, a_n, pairs, counts, starts, gather_rows, n_uniq = scatter_info
    yc = np.empty((n_uniq, D), np.float32)
    for g, (he, le) in enumerate(pairs):
        ya = results[2 * g]["yta"] + results[2 * g + 1]["yta"]   # [D, C1]
        yb = results[2 * g]["ytb"] + results[2 * g + 1]["ytb"]   # [D, C2]
        yc[starts[he]:starts[he] + counts[he]] = ya[:, :counts[he]].T
        yc[starts[le]:starts[le] + counts[le]] = yb[:, :counts[le]].T
    return yc[gather_rows].reshape(t_n, a_n, D)


def kernel(**inputs):
    from concourse.bass_utils import run_bass_kernel_spmd

    nc, in_maps, scatter_info = _prepare(inputs)
    res = run_bass_kernel_spmd(nc, in_maps, core_ids=list(range(N_CORES)))
    return _scatter(res.results, scatter_info)


# revision 5
# speedup vs baseline: 1.1698x; 1.0394x over previous
"""MoE ConditionalFeedForward kernel for 8 trn2 NeuronCores.

Strategy: expert parallelism with 2-way intermediate (I) splitting for load
balance. The 8 experts are ranked by routed-token count and paired
heaviest-with-lightest into 4 groups; the two cores of group g each own HALF
the I-rows (22 of 44 128-row blocks) of BOTH experts in the group and process
ALL tokens routed to them. Each core therefore streams exactly 44 block-rows
of w1/w3/w2 (the same HBM traffic as one full expert) while its matmul column
count is bounded by max-heavy + max-light counts instead of the global max.
Token (t, slot) pairs whose two slots hit the same expert are deduplicated
(computed once, scattered twice).

Per core, slice s in {heavy, light} with capacity C_s:
  phase 1: hT[i, c] = silu(sum_d w1T[d,i] xT[d,c]) * (sum_d w3T[d,i] xT[d,c])
           for the 22 owned i-blocks (PE matmuls, d on partitions)
  phase 2: yT[d, c] = sum_{i in owned} hT[i, c] * w2[i, d]  (partial sum)
The two partial yT of an expert are summed on the host (f32) and scattered.

All weights/activations stream as bf16; PSUM accumulation is f32.
"""

import numpy as np
import ml_dtypes

BF16 = ml_dtypes.bfloat16

# Problem dims (hardcoded per contract; kernel.py must be self-contained).
T, A, E, D, I = 1024, 2, 8, 2048, 5632
N_CORES = 8
DB = D // 128          # 16 d-chunks (phase-1 contraction)
IB = I // 128          # 44 i-blocks total per expert
HB = IB // 2           # 22 i-blocks per core slice
NPASS = 8              # phase-2 passes over D
W = D // NPASS         # 256 output columns per phase-2 pass
NDC = W // 128         # 2 128-col d-blocks per pass
GSZ = 8                # w2 i-blocks per DMA group

_BUILD_CACHE = {}


def _pad4(n):
    return max(4, -(-int(n) // 4) * 4)


def _build(C1, C2):
    """Build + compile the per-core Bass program for slice capacities C1, C2."""
    key = (C1, C2)
    if key in _BUILD_CACHE:
        return _BUILD_CACHE[key]

    import concourse.mybir as mybir
    import concourse.tile as tile
    from concourse import bacc

    dt = mybir.dt
    WDT = dt.bfloat16
    F32 = dt.float32

    assert C1 <= 512 and C2 <= C1

    nc = bacc.Bacc("TRN2", target_bir_lowering=False, debug=False,
                   num_devices=N_CORES)

    xa_t = nc.dram_tensor("xga", [128, DB * C1], WDT, kind="ExternalInput").ap()
    xb_t = nc.dram_tensor("xgb", [128, DB * C2], WDT, kind="ExternalInput").ap()
    w1a_t = nc.dram_tensor("w1a", [128, HB * DB * 128], WDT,
                           kind="ExternalInput").ap()
    w3a_t = nc.dram_tensor("w3a", [128, HB * DB * 128], WDT,
                           kind="ExternalInput").ap()
    w1b_t = nc.dram_tensor("w1b", [128, HB * DB * 128], WDT,
                           kind="ExternalInput").ap()
    w3b_t = nc.dram_tensor("w3b", [128, HB * DB * 128], WDT,
                           kind="ExternalInput").ap()
    w2a_t = nc.dram_tensor("w2a", [NPASS, 128, HB * W], WDT,
                           kind="ExternalInput").ap()
    w2b_t = nc.dram_tensor("w2b", [NPASS, 128, HB * W], WDT,
                           kind="ExternalInput").ap()
    # outputs are y transposed ([D, C]) partial sums; host adds + untransposes.
    ya_t = nc.dram_tensor("yta", [D, C1], F32, kind="ExternalOutput").ap()
    yb_t = nc.dram_tensor("ytb", [D, C2], F32, kind="ExternalOutput").ap()

    slices = [(C1, xa_t, w1a_t, w3a_t, w2a_t, ya_t),
              (C2, xb_t, w1b_t, w3b_t, w2b_t, yb_t)]

    with tile.TileContext(nc) as tc:
        with (
            tc.tile_pool(name="xpool", bufs=1) as xpool,
            tc.tile_pool(name="w1pool", bufs=5) as w1pool,
            tc.tile_pool(name="w3pool", bufs=5) as w3pool,
            # deep w2 prefetch: fills the DMA-idle trough at each phase-1
            # tail (w1/w3 fully delivered ~20us before phase 1 ends) so
            # phase 2 never starves at pass boundaries.
            tc.tile_pool(name="w2pool", bufs=12) as w2pool,
            tc.tile_pool(name="hpool", bufs=1) as hpool,
            tc.tile_pool(name="spool", bufs=2) as spool,
            tc.tile_pool(name="opool", bufs=4) as opool,
            tc.tile_pool(name="ps", bufs=2, space="PSUM") as ps,
        ):
            xg = {}
            for s, (C, x_t, *_r) in enumerate(slices):
                xg[s] = xpool.tile([128, DB * C], WDT, tag=f"x{s}",
                                   name=f"xg{s}")

            for s, (C, x_t, w1_t, w3_t, w2_t, y_t) in enumerate(slices):
                h = hpool.tile([128, HB * C], WDT, tag=f"h{s}")

                # ---- phase 1 ----
                # warmup: tiny first weight groups so the first matmul is
                # gated on ~0.25 MB of DMA instead of a full 1 MB tile.
                groups = ([1, 1] + [2] * 10) if s == 0 else [2] * 11
                b0 = 0
                for gi, nb in enumerate(groups):
                    wt1 = w1pool.tile([128, 2 * DB * 128], WDT, tag="w1")
                    wt3 = w3pool.tile([128, 2 * DB * 128], WDT, tag="w3")
                    span = nb * DB * 128
                    if s == 0 and gi == 0:
                        # startup-critical ordering: interleave half-group
                        # w1/w3 chunks with the first x chunk so the first
                        # ps1 chain gates on ~0.4 MB and the ps3 chain's
                        # weights land before it starts.
                        hspan = span // 2
                        nc.sync.dma_start(wt1[:, :hspan], w1_t[:, :hspan])
                        nc.sync.dma_start(xg[0][:, :2 * C], x_t[:, :2 * C])
                        nc.sync.dma_start(wt3[:, :hspan], w3_t[:, :hspan])
                        nc.sync.dma_start(wt1[:, hspan:span],
                                          w1_t[:, hspan:span])
                        nc.sync.dma_start(wt3[:, hspan:span],
                                          w3_t[:, hspan:span])
                        for q0 in range(2 * C, DB * C, 5 * C):
                            q1 = min(q0 + 5 * C, DB * C)
                            nc.sync.dma_start(xg[0][:, q0:q1], x_t[:, q0:q1])
                    else:
                        nc.sync.dma_start(
                            wt1[:, :span],
                            w1_t[:, b0 * DB * 128:(b0 + nb) * DB * 128])
                        nc.sync.dma_start(
                            wt3[:, :span],
                            w3_t[:, b0 * DB * 128:(b0 + nb) * DB * 128])
                    if s == 0 and gi == 6:
                        # slice-1 x, issued past the startup ramp so it
                        # streams during slice-0 compute (needed much later).
                        C2_, xb = slices[1][0], slices[1][1]
                        for q0 in range(0, DB * C2_, 8 * C2_):
                            q1 = min(q0 + 8 * C2_, DB * C2_)
                            nc.sync.dma_start(xg[1][:, q0:q1], xb[:, q0:q1])

                    for sb in range(nb):
                        b = b0 + sb
                        ps1 = ps.tile([128, C1], F32, tag="ps1")
                        ps3 = ps.tile([128, C1], F32, tag="ps3")
                        for do in range(DB):
                            lo = (sb * DB + do) * 128
                            nc.tensor.matmul(
                                ps1[:, :C], wt1[:, lo:lo + 128],
                                xg[s][:, do * C:(do + 1) * C],
                                start=(do == 0), stop=(do == DB - 1))
                        for do in range(DB):
                            lo = (sb * DB + do) * 128
                            nc.tensor.matmul(
                                ps3[:, :C], wt3[:, lo:lo + 128],
                                xg[s][:, do * C:(do + 1) * C],
                                start=(do == 0), stop=(do == DB - 1))
                        sig = spool.tile([128, C1], F32, tag="sig")
                        nc.scalar.activation(
                            sig[:, :C], ps1[:, :C],
                            mybir.ActivationFunctionType.Sigmoid)
                        m1 = spool.tile([128, C1], F32, tag="m1")
                        nc.vector.tensor_mul(m1[:, :C], sig[:, :C], ps3[:, :C])
                        nc.vector.tensor_mul(
                            h[:, b * C:(b + 1) * C], m1[:, :C], ps1[:, :C])
                    b0 += nb

                # ---- phase 2: yT[d, c] = sum_b w2[b, d].T @ hT[b, c] ----
                w2groups = [(g0, min(GSZ, HB - g0)) for g0 in range(0, HB, GSZ)]
                for ph in range(NPASS):
                    po = {}
                    for dc in range(NDC):
                        po[dc] = ps.tile([128, C1], F32, tag=f"y{dc}",
                                         name=f"po{dc}")
                    for g0, nb in w2groups:
                        wt2 = w2pool.tile([128, GSZ * W], WDT, tag="w2")
                        nc.sync.dma_start(wt2[:, :nb * W],
                                          w2_t[ph][:, g0 * W:(g0 + nb) * W])
                        for sb in range(nb):
                            b = g0 + sb
                            for dc in range(NDC):
                                lo = sb * W + dc * 128
                                nc.tensor.matmul(
                                    po[dc][:, :C],
                                    wt2[:, lo:lo + 128],
                                    h[:, b * C:(b + 1) * C],
                                    start=(b == 0), stop=(b == HB - 1))
                    for dc in range(NDC):
                        ot = opool.tile([128, C1], F32, tag="ot")
                        nc.vector.tensor_copy(ot[:, :C], po[dc][:, :C])
                        nc.scalar.dma_start(
                            y_t[ph * W + dc * 128:ph * W + dc * 128 + 128, :],
                            ot[:, :C])

    nc.compile()
    _BUILD_CACHE[key] = nc
    return nc


def _pack13(wh):
    """[2816, 2048] w1/w3 half -> phase-1 layout [128, HB*DB*128]:
    col = (b*DB + do)*128 + i_in, partition = d_in."""
    return np.ascontiguousarray(
        wh.reshape(HB, 128, DB, 128).transpose(3, 0, 2, 1)
    ).reshape(128, HB * DB * 128)


def _pack2(wh):
    """[2816, 2048] w2 half -> phase-2 layout [NPASS, 128, HB*W]:
    per pass, col = b*W + j, partition = i_in."""
    return np.ascontiguousarray(
        wh.reshape(HB, 128, NPASS, W).transpose(2, 1, 0, 3)
    ).reshape(NPASS, 128, HB * W)


def _packx(x_bf, tokens, C):
    """Gather token rows of x (bf16) and lay out as [128, DB*C]:
    col = do*C + c, partition = d_in."""
    xp = np.zeros((C, D), BF16)
    xp[:len(tokens)] = x_bf[tokens]
    return np.ascontiguousarray(
        xp.reshape(C, DB, 128).transpose(2, 1, 0)
    ).reshape(128, DB * C)


def _prepare(inputs):
    """Host routing + packing. Returns (nc, in_maps, scatter_info)."""
    x = np.asarray(inputs["x"])
    idx = np.asarray(inputs["expert_indices"])
    w1 = np.asarray(inputs["w1"])
    w2 = np.asarray(inputs["w2"])
    w3 = np.asarray(inputs["w3"])

    t_n, a_n = idx.shape

    # ---- dedup + routing ----
    tt = np.repeat(np.arange(t_n), a_n)
    ee = idx.reshape(-1).astype(np.int64)
    keys = tt * E + ee
    uniq = np.unique(keys)                        # sorted (t, e) pairs
    ue = uniq % E
    ut = uniq // E
    order = np.argsort(ue, kind="stable")         # grouped by expert
    counts = np.bincount(ue, minlength=E)
    starts = np.concatenate([[0], np.cumsum(counts)])
    # concat-layout row of each unique pair, and the gather map for scatter
    col = np.empty(len(uniq), np.int64)
    col[order] = np.arange(len(uniq)) - starts[ue[order]]
    concat_row = starts[ue] + col
    gather_rows = concat_row[np.searchsorted(uniq, keys)]   # [T*A]

    # ---- heavy/light pairing ----
    rank = np.argsort(-counts, kind="stable")
    pairs = [(int(rank[i]), int(rank[7 - i])) for i in range(4)]
    C1 = _pad4(counts[rank[0]])
    C2 = _pad4(counts[rank[4]])
    tokens_of = {
        int(e): ut[order[starts[e]:starts[e] + counts[e]]] for e in range(E)
    }

    nc = _build(C1, C2)

    x_bf = x.astype(BF16)
    w1_bf = {}
    in_maps = [dict() for _ in range(N_CORES)]
    for g, (he, le) in enumerate(pairs):
        xa = _packx(x_bf, tokens_of[he], C1)
        xb = _packx(x_bf, tokens_of[le], C2)
        for half in range(2):
            c = 2 * g + half
            r0, r1 = half * (I // 2), (half + 1) * (I // 2)
            in_maps[c]["xga"] = xa
            in_maps[c]["xgb"] = xb
            in_maps[c]["w1a"] = _pack13(w1[he][r0:r1].astype(BF16))
            in_maps[c]["w3a"] = _pack13(w3[he][r0:r1].astype(BF16))
            in_maps[c]["w2a"] = _pack2(w2[he][r0:r1].astype(BF16))
            in_maps[c]["w1b"] = _pack13(w1[le][r0:r1].astype(BF16))
            in_maps[c]["w3b"] = _pack13(w3[le][r0:r1].astype(BF16))
            in_maps[c]["w2b"] = _pack2(w2[le][r0:r1].astype(BF16))

    scatter_info = (t_n, a_n, pairs, counts, starts, gather_rows, len(uniq))
    return nc, in_maps, scatter_info


def _scatter(results, scatter_info):
    t_n, a_n, pairs, counts, starts, gather_rows, n_uniq = scatter_info
    yc = np.empty((n_uniq, D), np.float32)
    for g, (he, le) in enumerate(pairs):
        ya = results[2 * g]["yta"] + results[2 * g + 1]["yta"]   # [D, C1]
        yb = results[2 * g]["ytb"] + results[2 * g + 1]["ytb"]   # [D, C2]
        yc[starts[he]:starts[he] + counts[he]] = ya[:, :counts[he]].T
        yc[starts[le]:starts[le] + counts[le]] = yb[:, :counts[le]].T
    return yc[gather_rows].reshape(t_n, a_n, D)


def kernel(**inputs):
    from concourse.bass_utils import run_bass_kernel_spmd

    nc, in_maps, scatter_info = _prepare(inputs)
    res = run_bass_kernel_spmd(nc, in_maps, core_ids=list(range(N_CORES)))
    return _scatter(res.results, scatter_info)
